# revision 21
# baseline (speedup 1.0000x reference)
"""Trainium2 Bass kernel for nn_Model1_52518860096440 (dense_transformer).

Reference (B=4, S=4096, HID=1024, H=16, DH=64):
    qkv = query @ W_qkv.T + b_qkv           # only `query` is used
    q, k, v = split(qkv) -> (B,S,H,DH)
    s = einsum('bshd,bsgd->bshg', q, k) / 8 + attn_mask   # per-position 16x16
    p = softmax(s, -1); out = einsum('bshg,bsgd->bshd', p, v)

Strategy: 16384 tokens sharded 8 ways (2048/core, 16 tiles of 128).
Phase 2 runs almost entirely on the PE via per-token 16-row matmuls:
  - projection produces q/k in [d, h, t] layout (per-head columns), v
    token-major; v round-trips DRAM into [g, slot, d] replicas at
    partition offsets 32*(j%4).
  - scores: per-token matmul k_t[d,g]^T q_t[d,h] parked in PSUM at
    [32*(j%4)+g, 16*(j//4)+h]; softmax denominators via one static
    block-ones matmul (sums replicated); exp on ACT; normalize on DVE.
  - AV: per-token matmul vrep[g,d]^T e2s[g,h] parked at
    [64*(j%2)+d, 16*(j//2)+h]; raw parked layout is dumped and the host
    decodes it.
"""

from contextlib import ExitStack

import numpy as np

B, S, HID, H = 4, 4096, 1024, 16
DH = HID // H                 # 64
NCORES = 8
T = B * S                     # 16384 tokens
TC = T // NCORES              # 2048 tokens per core
P = 128
NT = TC // P                  # 16 token tiles per core
KT = HID // P                 # 8 contraction chunks
NEG = -30000.0                # mask fill for dead partition rows

_compiled = {}


def _build(phase=4):
    import concourse.tile as tile
    import concourse.mybir as mybir
    from concourse import bacc

    f32 = mybir.dt.float32
    f16 = mybir.dt.float16
    Act = mybir.ActivationFunctionType

    nc = bacc.Bacc("TRN2", target_bir_lowering=False, debug=False,
                   num_devices=NCORES)

    xk_d = nc.dram_tensor("xk", (NT, P, KT, P), f16, kind="ExternalInput")
    wqk_d = nc.dram_tensor("wqk", (P, H, KT, P), f16, kind="ExternalInput")
    bqk_d = nc.dram_tensor("bqk", (1, H, P), f16, kind="ExternalInput")
    bv_d = nc.dram_tensor("bv", (1, HID), f16, kind="ExternalInput")
    wv_d = nc.dram_tensor("wv", (P, KT, HID), f16, kind="ExternalInput")
    m2_d = nc.dram_tensor("m2", (NT, P, 512), f16, kind="ExternalInput")
    obd_d = nc.dram_tensor("obd", (P, P), f16, kind="ExternalInput")
    out_d = nc.dram_tensor("out", (NT, P, HID), f16, kind="ExternalOutput")

    with tile.TileContext(nc) as tc, ExitStack() as ctx:
        const = ctx.enter_context(tc.tile_pool(name="const", bufs=1))
        xpool = ctx.enter_context(tc.tile_pool(name="x", bufs=2))
        qkpool = ctx.enter_context(tc.tile_pool(name="qk", bufs=2))
        vpool = ctx.enter_context(tc.tile_pool(name="v", bufs=2))
        vreppool = ctx.enter_context(tc.tile_pool(name="vrep", bufs=3))
        mpool = ctx.enter_context(tc.tile_pool(name="m", bufs=3))
        epool = ctx.enter_context(tc.tile_pool(name="e", bufs=2))
        opool = ctx.enter_context(tc.tile_pool(name="o", bufs=2))
        dpool = ctx.enter_context(tc.tile_pool(name="dscr", bufs=2,
                                               space="DRAM"))
        psq = ctx.enter_context(tc.tile_pool(name="psq", bufs=2, space="PSUM"))
        psv = ctx.enter_context(tc.tile_pool(name="psv", bufs=2, space="PSUM"))
        pss = ctx.enter_context(tc.tile_pool(name="pss", bufs=1, space="PSUM"))
        pssum = ctx.enter_context(tc.tile_pool(name="pssum", bufs=1,
                                               space="PSUM"))
        psav = ctx.enter_context(tc.tile_pool(name="psav", bufs=1,
                                              space="PSUM"))

        # ---- resident constants ----
        # first head-group weights + first tile's x load before the bulk so
        # the PE can start immediately
        wqk_sb = const.tile([P, H, KT, P], f16)
        nc.sync.dma_start(wqk_sb[:, 0, :, :], wqk_d[:, 0, :, :])
        xk0 = xpool.tile([P, KT, P], f16, tag="xk")
        nc.sync.dma_start(xk0[:], xk_d[0])
        for h in range(1, 4):
            nc.sync.dma_start(wqk_sb[:, h, :, :], wqk_d[:, h, :, :])
        bqk_sb = const.tile([1, H, P], f16, tag="bqk")
        nc.sync.dma_start(bqk_sb[:], bqk_d[:])
        ones_row = const.tile([1, P], f16, tag="ones_row")
        nc.vector.memset(ones_row[:], 1.0)
        for h in range(4, H):
            nc.sync.dma_start(wqk_sb[:, h, :, :], wqk_d[:, h, :, :])
        bv_sb = const.tile([1, HID], f16, tag="bv")
        nc.sync.dma_start(bv_sb[:], bv_d[:])
        wv_sb = const.tile([P, KT, HID], f16)
        for kt in range(KT):
            nc.sync.dma_start(wv_sb[:, kt, :], wv_d[:, kt, :])
        obd_sb = const.tile([P, P], f16)
        nc.sync.dma_start(obd_sb[:], obd_d[:])
        neg2 = const.tile([P, 1], f32, tag="neg2")
        nc.vector.memset(neg2[:], -2.0)

        # persistent scores psum bank; dead rows zeroed once
        sps = pss.tile([P, 512], f32)
        nc.vector.memset(sps[:], 0.0)

        # per-iteration state carried between pipeline stages
        st = {}

        def stage_a(t):
            """Projection for tile t: q/k per-head layout + v roundtrip."""
            if t == 0:
                xk = xk0
            else:
                xk = xpool.tile([P, KT, P], f16, tag="xk")
                nc.sync.dma_start(xk[:], xk_d[t])
            m2 = mpool.tile([P, 512], f16, tag="m2")
            nc.sync.dma_start(m2[:], m2_d[t])

            q_sb = qkpool.tile([64, H, P], f16, tag="q")
            k_sb = qkpool.tile([64, H, P], f16, tag="k")
            for hg in range(4):
                ps = psq.tile([P, 512], f32, tag="qkps")
                for hh in range(4):
                    h = hg * 4 + hh
                    osl = slice(hh * P, (hh + 1) * P)
                    for kt in range(KT):
                        nc.tensor.matmul(ps[:, osl], wqk_sb[:, h, kt, :],
                                         xk[:, kt, :],
                                         start=(kt == 0), stop=False)
                    nc.tensor.matmul(ps[:, osl], bqk_sb[0:1, h, :],
                                     ones_row[0:1, :], start=False, stop=True)
                hsl = slice(hg * 4, (hg + 1) * 4)
                src_q = ps[0:64, :].rearrange("p (hh t) -> p hh t", t=P)
                src_k = ps[64:128, :].rearrange("p (hh t) -> p hh t", t=P)
                if hg % 2 == 0:
                    nc.scalar.copy(q_sb[:, hsl, :], src_q)
                    nc.scalar.copy(k_sb[:, hsl, :], src_k)
                else:
                    nc.vector.tensor_copy(q_sb[:, hsl, :], src_q)
                    nc.vector.tensor_copy(k_sb[:, hsl, :], src_k)

            v_sb = vpool.tile([P, HID], f16, tag="vsb")
            for oc in range(2):
                vps = psv.tile([P, 512], f32, tag="vps")
                osl = slice(oc * 512, (oc + 1) * 512)
                for kt in range(KT):
                    nc.tensor.matmul(vps[:], xk[:, kt, :],
                                     wv_sb[:, kt, osl],
                                     start=(kt == 0), stop=False)
                nc.tensor.matmul(vps[:], ones_row[0:1, :], bv_sb[0:1, osl],
                                 start=False, stop=True)
                nc.scalar.copy(v_sb[:, osl], vps[:])

            v_scr = dpool.tile([P, H, DH], f16, tag="vscr")
            nc.scalar.dma_start(v_scr[:],
                                v_sb[:].rearrange("t (g d) -> t g d", d=DH))
            vrep = vreppool.tile([P, 32, DH], f16, tag="vrep")
            vsrc = v_scr[:].rearrange("(s j4) g d -> j4 g s d", j4=4)
            for r in range(4):
                nc.scalar.dma_start(vrep[32 * r: 32 * r + 16, :, :], vsrc[r])
            st[t] = (q_sb, k_sb, vrep, m2)

        def stage_b1(t, part=None):
            """Scores + exp (part 'a') and softmax chain (part 'b')."""
            if part == "b":
                _stage_b1b(t)
                return
            q_sb, k_sb, vrep, m2 = st[t]
            if phase <= 1:
                o_sb = opool.tile([P, HID], f16, tag="osb")
                nc.vector.tensor_copy(o_sb[:, 0:512],
                                      vrep[:].rearrange("p s d -> p (s d)")[:, 0:512])
                nc.vector.tensor_copy(o_sb[:, 512:1024], m2[:])
                nc.sync.dma_start(out_d[t], o_sb[:])
                st[t] = (vrep, None)
                return
            for j4 in range(4):
                for slot in range(32):
                    j = slot * 4 + j4
                    nc.tensor.matmul(
                        sps[32 * j4: 32 * j4 + H, 16 * slot: 16 * slot + H],
                        k_sb[:, :, j], q_sb[:, :, j], start=True, stop=True,
                        tile_position=(0, 32 * j4))
            sm = epool.tile([P, 512], f32, tag="sm")
            nc.vector.tensor_add(sm[:], sps[:], m2[:])
            e2 = epool.tile([P, 512], f16, tag="e2")
            nc.scalar.activation(e2[:], sm[:], Act.Exp, bias=neg2[:])
            st[t] = (vrep, e2)
            if part is None:
                _stage_b1b(t)
                return
            return

        def _stage_b1b(t):
            vrep, e2 = st[t]
            sums = pssum.tile([P, 512], f32, tag="sums")
            nc.tensor.matmul(sums[:], obd_sb[:], e2[:], start=True, stop=True)
            r2 = epool.tile([P, 512], f32, tag="r2")
            nc.vector.reciprocal(r2[:], sums[:])
            e2s = epool.tile([P, 512], f16, tag="e2s")
            nc.vector.tensor_mul(e2s[:], e2[:], r2[:])
            if phase <= 2:
                o_sb = opool.tile([P, HID], f16, tag="osb")
                nc.vector.tensor_copy(o_sb[:, 0:512], e2[:])
                nc.vector.tensor_copy(o_sb[:, 512:1024], sm[:])
                nc.sync.dma_start(out_d[t], o_sb[:])
                st[t] = (vrep, None)
                return
            st[t] = (vrep, e2s)

        def stage_b2(t):
            """AV + output for tile t."""
            vrep, e2s = st.pop(t)
            if e2s is None:
                return
            if phase <= 3:
                o_sb = opool.tile([P, HID], f16, tag="osb")
                nc.vector.tensor_copy(o_sb[:, 0:512], e2s[:])
                nc.vector.tensor_copy(o_sb[:, 512:1024], e2s[:])
                nc.sync.dma_start(out_d[t], o_sb[:])
                return

            avps_a = psav.tile([P, 512], f32, tag="avps_a")
            avps_b = psav.tile([P, 512], f32, tag="avps_b")
            for j4 in range(4):
                for slot in range(32):
                    j = slot * 4 + j4
                    bank = avps_a if j < 64 else avps_b
                    col = ((j // 2) % 32) * 16
                    nc.tensor.matmul(
                        bank[64 * (j % 2): 64 * (j % 2) + DH, col: col + H],
                        vrep[32 * j4: 32 * j4 + H, slot, :],
                        e2s[32 * j4: 32 * j4 + H, 16 * slot: 16 * slot + H],
                        start=True, stop=True,
                        tile_position=(32 * j4, 64 * (j % 2)))
            o_sb = opool.tile([P, HID], f16, tag="osb")
            if phase <= 3.5:
                nc.vector.tensor_copy(o_sb[:, 0:512], e2s[:])
                nc.vector.tensor_copy(o_sb[:, 512:1024], e2s[:])
            else:
                nc.scalar.copy(o_sb[:, 0:512], avps_a[:])
                nc.scalar.copy(o_sb[:, 512:1024], avps_b[:])
            nc.scalar.dma_start(out_d[t], o_sb[:])

        for t in range(NT):
            stage_a(t)
            if t >= 1:
                stage_b1(t - 1)
            if t >= 2:
                stage_b2(t - 2)
        stage_b1(NT - 1, part="a")
        stage_b2(NT - 2)
        stage_b1(NT - 1, part="b")
        stage_b2(NT - 1)

    nc.compile()
    return nc


def _host_prep(query, W_qkv, b_qkv, attn_mask):
    scale = 1.0 / np.sqrt(DH)
    x = np.asarray(query, dtype=np.float32).reshape(T, HID)
    W = np.asarray(W_qkv, dtype=np.float32)
    b = np.asarray(b_qkv, dtype=np.float32)
    m = np.asarray(attn_mask, dtype=np.float32).reshape(T, H, H)

    # wqk[kp, h, kt, sel*64+d]
    Wq = (W[0:HID] * scale).reshape(H, DH, KT, P)      # [h, d, kt, kp]
    Wk = W[HID:2 * HID].reshape(H, DH, KT, P)
    wqk = np.stack([Wq, Wk], axis=0)                   # [sel, h, d, kt, kp]
    wqk = np.ascontiguousarray(
        wqk.transpose(4, 1, 3, 0, 2).reshape(P, H, KT, P)).astype(np.float16)
    bq = (b[0:HID] * scale).reshape(H, DH)
    bk = b[HID:2 * HID].reshape(H, DH)
    bqk = np.stack([bq, bk], axis=1).reshape(1, H, P).astype(np.float16)
    bv = b[2 * HID:].reshape(1, HID).astype(np.float16)

    # wv[kp, kt, o]
    wv = np.ascontiguousarray(
        W[2 * HID:].reshape(HID, KT, P).transpose(2, 1, 0)).astype(np.float16)

    # ones_bd: block r rows 32r..32r+15 (g), cols 32r..32r+31
    obd = np.zeros((P, P), dtype=np.float16)
    for r in range(4):
        obd[32 * r: 32 * r + H, 32 * r: 32 * r + 32] = 1.0

    # per-core xk and mask2
    xks, m2s = [], []
    for c in range(NCORES):
        xc = x[c * TC:(c + 1) * TC].reshape(NT, P, KT, P)   # [t, j, kt, kp]
        xks.append(np.ascontiguousarray(
            xc.transpose(0, 3, 2, 1)).astype(np.float16))   # [t, kp, kt, j]
        mc = m[c * TC:(c + 1) * TC].reshape(NT, 32, 4, H, H)  # [t,slot,j4,h,g]
        m2 = np.full((NT, 4, 32, 32, H), NEG, dtype=np.float32)
        m2[:, :, 0:H, :, :] = mc.transpose(0, 2, 4, 1, 3)   # [t, j4, g, slot, h]
        m2s.append(m2.reshape(NT, P, 512).astype(np.float16))
    return xks, wqk, bqk, bv, wv, m2s, obd


def kernel(query, key, value, attn_mask, W_qkv, b_qkv):
    from concourse.bass_utils import run_bass_kernel_spmd

    xks, wqk, bqk, bv, wv, m2s, obd = _host_prep(query, W_qkv, b_qkv,
                                                 attn_mask)

    if "nc" not in _compiled:
        _compiled["nc"] = _build()
    nc = _compiled["nc"]

    in_maps = []
    for c in range(NCORES):
        in_maps.append({
            "xk": xks[c], "wqk": wqk, "bqk": bqk, "bv": bv, "wv": wv,
            "m2": m2s[c], "obd": obd,
        })

    res = run_bass_kernel_spmd(nc, in_maps, core_ids=list(range(NCORES)))

    # decode parked output: arr[t, 64*(j%2)+d, 16*(j//2)+h]
    outs = []
    for c in range(NCORES):
        arr = np.asarray(res.results[c]["out"], dtype=np.float32)
        arr = arr.reshape(NT, 2, DH, 64, H)          # [t, j2, d, jh, h]
        o = arr.transpose(0, 3, 1, 4, 2).reshape(TC, HID)
        outs.append(o)
    out = np.concatenate(outs, axis=0)
    return out.reshape(B, S, HID).astype(np.float32)


if __name__ == "__main__":
    rng = np.random.default_rng(0)
    inputs = {
        "query": rng.standard_normal((B, S, HID), dtype=np.float32),
        "key": rng.standard_normal((B, S, HID), dtype=np.float32),
        "value": rng.standard_normal((B, S, HID), dtype=np.float32),
        "attn_mask": rng.standard_normal((B, S, H, H), dtype=np.float32),
        "W_qkv": (rng.standard_normal((3 * HID, HID), dtype=np.float32)
                  / np.sqrt(HID)),
        "b_qkv": rng.standard_normal((3 * HID,), dtype=np.float32) * 0.01,
    }
    out = kernel(**inputs)

    # numpy reference
    x = inputs["query"].reshape(T, HID)
    qkv = x @ inputs["W_qkv"].T + inputs["b_qkv"]
    q, k, v = np.split(qkv, 3, axis=-1)
    q = q.reshape(T, H, DH) / np.sqrt(DH)
    k = k.reshape(T, H, DH)
    v = v.reshape(T, H, DH)
    s = np.einsum("thd,tgd->thg", q, k) + inputs["attn_mask"].reshape(T, H, H)
    p = np.exp(s - s.max(-1, keepdims=True))
    p /= p.sum(-1, keepdims=True)
    o = np.einsum("thg,tgd->thd", p, v).reshape(B, S, HID)
    err = np.abs(out - o).max()
    print("kernel output:", out.shape, "abs err:", err,
          "rel:", err / np.abs(o).max())


# revision 22
# speedup vs baseline: 1.0003x; 1.0003x over previous
"""Trainium2 Bass kernel for nn_Model1_52518860096440 (dense_transformer).

Reference (B=4, S=4096, HID=1024, H=16, DH=64):
    qkv = query @ W_qkv.T + b_qkv           # only `query` is used
    q, k, v = split(qkv) -> (B,S,H,DH)
    s = einsum('bshd,bsgd->bshg', q, k) / 8 + attn_mask   # per-position 16x16
    p = softmax(s, -1); out = einsum('bshg,bsgd->bshd', p, v)

Strategy: 16384 tokens sharded 8 ways (2048/core, 16 tiles of 128).
Phase 2 runs almost entirely on the PE via per-token 16-row matmuls:
  - projection produces q/k in [d, h, t] layout (per-head columns), v
    token-major; v round-trips DRAM into [g, slot, d] replicas at
    partition offsets 32*(j%4).
  - scores: per-token matmul k_t[d,g]^T q_t[d,h] parked in PSUM at
    [32*(j%4)+g, 16*(j//4)+h]; softmax denominators via one static
    block-ones matmul (sums replicated); exp on ACT; normalize on DVE.
  - AV: per-token matmul vrep[g,d]^T e2s[g,h] parked at
    [64*(j%2)+d, 16*(j//2)+h]; raw parked layout is dumped and the host
    decodes it.
"""

from contextlib import ExitStack

import numpy as np

B, S, HID, H = 4, 4096, 1024, 16
DH = HID // H                 # 64
NCORES = 8
T = B * S                     # 16384 tokens
TC = T // NCORES              # 2048 tokens per core
P = 128
NT = TC // P                  # 16 token tiles per core
KT = HID // P                 # 8 contraction chunks
NEG = -30000.0                # mask fill for dead partition rows

_compiled = {}


def _build(phase=4):
    import concourse.tile as tile
    import concourse.mybir as mybir
    from concourse import bacc

    f32 = mybir.dt.float32
    f16 = mybir.dt.float16
    Act = mybir.ActivationFunctionType

    nc = bacc.Bacc("TRN2", target_bir_lowering=False, debug=False,
                   num_devices=NCORES)

    xk_d = nc.dram_tensor("xk", (NT, P, KT, P), f16, kind="ExternalInput")
    wqk_d = nc.dram_tensor("wqk", (P, H, KT, P), f16, kind="ExternalInput")
    bqk_d = nc.dram_tensor("bqk", (1, H, P), f16, kind="ExternalInput")
    bv_d = nc.dram_tensor("bv", (1, HID), f16, kind="ExternalInput")
    wv_d = nc.dram_tensor("wv", (P, KT, HID), f16, kind="ExternalInput")
    m2_d = nc.dram_tensor("m2", (NT, P, 512), f16, kind="ExternalInput")
    obd_d = nc.dram_tensor("obd", (P, P), f16, kind="ExternalInput")
    out_d = nc.dram_tensor("out", (NT, P, HID), f16, kind="ExternalOutput")

    with tile.TileContext(nc) as tc, ExitStack() as ctx:
        const = ctx.enter_context(tc.tile_pool(name="const", bufs=1))
        xpool = ctx.enter_context(tc.tile_pool(name="x", bufs=2))
        qkpool = ctx.enter_context(tc.tile_pool(name="qk", bufs=2))
        vpool = ctx.enter_context(tc.tile_pool(name="v", bufs=2))
        vreppool = ctx.enter_context(tc.tile_pool(name="vrep", bufs=3))
        mpool = ctx.enter_context(tc.tile_pool(name="m", bufs=3))
        epool = ctx.enter_context(tc.tile_pool(name="e", bufs=2))
        opool = ctx.enter_context(tc.tile_pool(name="o", bufs=2))
        dpool = ctx.enter_context(tc.tile_pool(name="dscr", bufs=2,
                                               space="DRAM"))
        psq = ctx.enter_context(tc.tile_pool(name="psq", bufs=2, space="PSUM"))
        psv = ctx.enter_context(tc.tile_pool(name="psv", bufs=2, space="PSUM"))
        pss = ctx.enter_context(tc.tile_pool(name="pss", bufs=1, space="PSUM"))
        pssum = ctx.enter_context(tc.tile_pool(name="pssum", bufs=1,
                                               space="PSUM"))
        psav = ctx.enter_context(tc.tile_pool(name="psav", bufs=1,
                                              space="PSUM"))

        # ---- resident constants ----
        # first head-group weights + first tile's x load before the bulk so
        # the PE can start immediately
        wqk_sb = const.tile([P, H, KT, P], f16)
        nc.sync.dma_start(wqk_sb[:, 0, :, :], wqk_d[:, 0, :, :])
        xk0 = xpool.tile([P, KT, P], f16, tag="xk")
        nc.sync.dma_start(xk0[:], xk_d[0])
        for h in range(1, 4):
            nc.sync.dma_start(wqk_sb[:, h, :, :], wqk_d[:, h, :, :])
        bqk_sb = const.tile([1, H, P], f16, tag="bqk")
        nc.sync.dma_start(bqk_sb[:], bqk_d[:])
        ones_row = const.tile([1, P], f16, tag="ones_row")
        nc.vector.memset(ones_row[:], 1.0)
        for h in range(4, H):
            nc.sync.dma_start(wqk_sb[:, h, :, :], wqk_d[:, h, :, :])
        bv_sb = const.tile([1, HID], f16, tag="bv")
        nc.sync.dma_start(bv_sb[:], bv_d[:])
        wv_sb = const.tile([P, KT, HID], f16)
        for kt in range(KT):
            nc.sync.dma_start(wv_sb[:, kt, :], wv_d[:, kt, :])
        obd_sb = const.tile([P, P], f16)
        nc.sync.dma_start(obd_sb[:], obd_d[:])
        neg2 = const.tile([P, 1], f32, tag="neg2")
        nc.vector.memset(neg2[:], -2.0)

        # persistent scores psum bank; dead rows zeroed once
        sps = pss.tile([P, 512], f32)
        nc.vector.memset(sps[:], 0.0)

        # per-iteration state carried between pipeline stages
        st = {}

        def stage_a(t):
            """Projection for tile t: q/k per-head layout + v roundtrip."""
            if t == 0:
                xk = xk0
            else:
                xk = xpool.tile([P, KT, P], f16, tag="xk")
                nc.sync.dma_start(xk[:], xk_d[t])
            m2 = mpool.tile([P, 512], f16, tag="m2")
            nc.sync.dma_start(m2[:], m2_d[t])

            q_sb = qkpool.tile([64, H, P], f16, tag="q")
            k_sb = qkpool.tile([64, H, P], f16, tag="k")
            for hg in range(4):
                ps = psq.tile([P, 512], f32, tag="qkps")
                for hh in range(4):
                    h = hg * 4 + hh
                    osl = slice(hh * P, (hh + 1) * P)
                    for kt in range(KT):
                        nc.tensor.matmul(ps[:, osl], wqk_sb[:, h, kt, :],
                                         xk[:, kt, :],
                                         start=(kt == 0), stop=False)
                    nc.tensor.matmul(ps[:, osl], bqk_sb[0:1, h, :],
                                     ones_row[0:1, :], start=False, stop=True)
                hsl = slice(hg * 4, (hg + 1) * 4)
                src_q = ps[0:64, :].rearrange("p (hh t) -> p hh t", t=P)
                src_k = ps[64:128, :].rearrange("p (hh t) -> p hh t", t=P)
                if hg % 2 == 0:
                    nc.scalar.copy(q_sb[:, hsl, :], src_q)
                    nc.scalar.copy(k_sb[:, hsl, :], src_k)
                else:
                    nc.vector.tensor_copy(q_sb[:, hsl, :], src_q)
                    nc.vector.tensor_copy(k_sb[:, hsl, :], src_k)

            v_sb = vpool.tile([P, HID], f16, tag="vsb")
            for oc in range(2):
                vps = psv.tile([P, 512], f32, tag="vps")
                osl = slice(oc * 512, (oc + 1) * 512)
                for kt in range(KT):
                    nc.tensor.matmul(vps[:], xk[:, kt, :],
                                     wv_sb[:, kt, osl],
                                     start=(kt == 0), stop=False)
                nc.tensor.matmul(vps[:], ones_row[0:1, :], bv_sb[0:1, osl],
                                 start=False, stop=True)
                nc.scalar.copy(v_sb[:, osl], vps[:])

            v_scr = dpool.tile([P, H, DH], f16, tag="vscr")
            nc.scalar.dma_start(v_scr[:],
                                v_sb[:].rearrange("t (g d) -> t g d", d=DH))
            vrep = vreppool.tile([P, 32, DH], f16, tag="vrep")
            vsrc = v_scr[:].rearrange("(s j4) g d -> j4 g s d", j4=4)
            for r in range(4):
                nc.scalar.dma_start(vrep[32 * r: 32 * r + 16, :, :], vsrc[r])
            st[t] = (q_sb, k_sb, vrep, m2)

        def stage_b1(t, part=None):
            """Scores + exp (part 'a') and softmax chain (part 'b')."""
            if part == "b":
                _stage_b1b(t)
                return
            q_sb, k_sb, vrep, m2 = st[t]
            if phase <= 1:
                o_sb = opool.tile([P, HID], f16, tag="osb")
                nc.vector.tensor_copy(o_sb[:, 0:512],
                                      vrep[:].rearrange("p s d -> p (s d)")[:, 0:512])
                nc.vector.tensor_copy(o_sb[:, 512:1024], m2[:])
                nc.sync.dma_start(out_d[t], o_sb[:])
                st[t] = (vrep, None)
                return
            for j4 in range(4):
                for slot in range(32):
                    j = slot * 4 + j4
                    nc.tensor.matmul(
                        sps[32 * j4: 32 * j4 + H, 16 * slot: 16 * slot + H],
                        k_sb[:, :, j], q_sb[:, :, j], start=True, stop=True,
                        tile_position=(0, 32 * j4))
            sm = epool.tile([P, 512], f32, tag="sm")
            nc.vector.tensor_add(sm[:], sps[:], m2[:])
            e2 = epool.tile([P, 512], f16, tag="e2")
            nc.scalar.activation(e2[:], sm[:], Act.Exp, bias=neg2[:])
            st[t] = (vrep, e2)
            if part is None:
                _stage_b1b(t)
                return
            return

        def _stage_b1b(t):
            vrep, e2 = st[t]
            sums = pssum.tile([P, 512], f32, tag="sums")
            nc.tensor.matmul(sums[:], obd_sb[:], e2[:], start=True, stop=True)
            r2 = epool.tile([P, 512], f32, tag="r2")
            nc.vector.reciprocal(r2[:], sums[:])
            e2s = epool.tile([P, 512], f16, tag="e2s")
            nc.vector.tensor_mul(e2s[:], e2[:], r2[:])
            if phase <= 2:
                o_sb = opool.tile([P, HID], f16, tag="osb")
                nc.vector.tensor_copy(o_sb[:, 0:512], e2[:])
                nc.vector.tensor_copy(o_sb[:, 512:1024], sm[:])
                nc.sync.dma_start(out_d[t], o_sb[:])
                st[t] = (vrep, None)
                return
            st[t] = (vrep, e2s)

        def stage_b2(t):
            """AV + output for tile t."""
            vrep, e2s = st.pop(t)
            if e2s is None:
                return
            if phase <= 3:
                o_sb = opool.tile([P, HID], f16, tag="osb")
                nc.vector.tensor_copy(o_sb[:, 0:512], e2s[:])
                nc.vector.tensor_copy(o_sb[:, 512:1024], e2s[:])
                nc.sync.dma_start(out_d[t], o_sb[:])
                return

            avps_a = psav.tile([P, 512], f32, tag="avps_a")
            avps_b = psav.tile([P, 512], f32, tag="avps_b")
            for j4 in range(4):
                for slot in range(32):
                    j = slot * 4 + j4
                    bank = avps_a if j < 64 else avps_b
                    col = ((j // 2) % 32) * 16
                    nc.tensor.matmul(
                        bank[64 * (j % 2): 64 * (j % 2) + DH, col: col + H],
                        vrep[32 * j4: 32 * j4 + H, slot, :],
                        e2s[32 * j4: 32 * j4 + H, 16 * slot: 16 * slot + H],
                        start=True, stop=True,
                        tile_position=(32 * j4, 64 * (j % 2)))
            o_sb = opool.tile([P, HID], f16, tag="osb")
            if phase <= 3.5:
                nc.vector.tensor_copy(o_sb[:, 0:512], e2s[:])
                nc.vector.tensor_copy(o_sb[:, 512:1024], e2s[:])
            else:
                nc.scalar.copy(o_sb[:, 0:512], avps_a[:])
                nc.scalar.copy(o_sb[:, 512:1024], avps_b[:])
            nc.sync.dma_start(out_d[t], o_sb[:])

        for t in range(NT):
            stage_a(t)
            if t >= 1:
                stage_b1(t - 1)
            if t >= 2:
                stage_b2(t - 2)
        stage_b1(NT - 1, part="a")
        stage_b2(NT - 2)
        stage_b1(NT - 1, part="b")
        stage_b2(NT - 1)

    nc.compile()
    return nc


def _host_prep(query, W_qkv, b_qkv, attn_mask):
    scale = 1.0 / np.sqrt(DH)
    x = np.asarray(query, dtype=np.float32).reshape(T, HID)
    W = np.asarray(W_qkv, dtype=np.float32)
    b = np.asarray(b_qkv, dtype=np.float32)
    m = np.asarray(attn_mask, dtype=np.float32).reshape(T, H, H)

    # wqk[kp, h, kt, sel*64+d]
    Wq = (W[0:HID] * scale).reshape(H, DH, KT, P)      # [h, d, kt, kp]
    Wk = W[HID:2 * HID].reshape(H, DH, KT, P)
    wqk = np.stack([Wq, Wk], axis=0)                   # [sel, h, d, kt, kp]
    wqk = np.ascontiguousarray(
        wqk.transpose(4, 1, 3, 0, 2).reshape(P, H, KT, P)).astype(np.float16)
    bq = (b[0:HID] * scale).reshape(H, DH)
    bk = b[HID:2 * HID].reshape(H, DH)
    bqk = np.stack([bq, bk], axis=1).reshape(1, H, P).astype(np.float16)
    bv = b[2 * HID:].reshape(1, HID).astype(np.float16)

    # wv[kp, kt, o]
    wv = np.ascontiguousarray(
        W[2 * HID:].reshape(HID, KT, P).transpose(2, 1, 0)).astype(np.float16)

    # ones_bd: block r rows 32r..32r+15 (g), cols 32r..32r+31
    obd = np.zeros((P, P), dtype=np.float16)
    for r in range(4):
        obd[32 * r: 32 * r + H, 32 * r: 32 * r + 32] = 1.0

    # per-core xk and mask2
    xks, m2s = [], []
    for c in range(NCORES):
        xc = x[c * TC:(c + 1) * TC].reshape(NT, P, KT, P)   # [t, j, kt, kp]
        xks.append(np.ascontiguousarray(
            xc.transpose(0, 3, 2, 1)).astype(np.float16))   # [t, kp, kt, j]
        mc = m[c * TC:(c + 1) * TC].reshape(NT, 32, 4, H, H)  # [t,slot,j4,h,g]
        m2 = np.full((NT, 4, 32, 32, H), NEG, dtype=np.float32)
        m2[:, :, 0:H, :, :] = mc.transpose(0, 2, 4, 1, 3)   # [t, j4, g, slot, h]
        m2s.append(m2.reshape(NT, P, 512).astype(np.float16))
    return xks, wqk, bqk, bv, wv, m2s, obd


def kernel(query, key, value, attn_mask, W_qkv, b_qkv):
    from concourse.bass_utils import run_bass_kernel_spmd

    xks, wqk, bqk, bv, wv, m2s, obd = _host_prep(query, W_qkv, b_qkv,
                                                 attn_mask)

    if "nc" not in _compiled:
        _compiled["nc"] = _build()
    nc = _compiled["nc"]

    in_maps = []
    for c in range(NCORES):
        in_maps.append({
            "xk": xks[c], "wqk": wqk, "bqk": bqk, "bv": bv, "wv": wv,
            "m2": m2s[c], "obd": obd,
        })

    res = run_bass_kernel_spmd(nc, in_maps, core_ids=list(range(NCORES)))

    # decode parked output: arr[t, 64*(j%2)+d, 16*(j//2)+h]
    outs = []
    for c in range(NCORES):
        arr = np.asarray(res.results[c]["out"], dtype=np.float32)
        arr = arr.reshape(NT, 2, DH, 64, H)          # [t, j2, d, jh, h]
        o = arr.transpose(0, 3, 1, 4, 2).reshape(TC, HID)
        outs.append(o)
    out = np.concatenate(outs, axis=0)
    return out.reshape(B, S, HID).astype(np.float32)


if __name__ == "__main__":
    rng = np.random.default_rng(0)
    inputs = {
        "query": rng.standard_normal((B, S, HID), dtype=np.float32),
        "key": rng.standard_normal((B, S, HID), dtype=np.float32),
        "value": rng.standard_normal((B, S, HID), dtype=np.float32),
        "attn_mask": rng.standard_normal((B, S, H, H), dtype=np.float32),
        "W_qkv": (rng.standard_normal((3 * HID, HID), dtype=np.float32)
                  / np.sqrt(HID)),
        "b_qkv": rng.standard_normal((3 * HID,), dtype=np.float32) * 0.01,
    }
    out = kernel(**inputs)

    # numpy reference
    x = inputs["query"].reshape(T, HID)
    qkv = x @ inputs["W_qkv"].T + inputs["b_qkv"]
    q, k, v = np.split(qkv, 3, axis=-1)
    q = q.reshape(T, H, DH) / np.sqrt(DH)
    k = k.reshape(T, H, DH)
    v = v.reshape(T, H, DH)
    s = np.einsum("thd,tgd->thg", q, k) + inputs["attn_mask"].reshape(T, H, H)
    p = np.exp(s - s.max(-1, keepdims=True))
    p /= p.sum(-1, keepdims=True)
    o = np.einsum("thg,tgd->thd", p, v).reshape(B, S, HID)
    err = np.abs(out - o).max()
    print("kernel output:", out.shape, "abs err:", err,
          "rel:", err / np.abs(o).max())
